# revision 25
# baseline (speedup 1.0000x reference)
"""Trainium2 Bass kernel for nn_AbsoluteAttention (XLNet-style attention with
softmax over the HEAD axis n, faithful to the source module).

Reference math (fp32):
    s[i,j,b,n]  = (sum_d q[i,b,n,d] k[j,b,n,d]) * 0.125
    s          -= 1e30 * mask[i,j,b,1]          (broadcast over n!)
    p           = softmax over n (axis -1)
    out[i,b,n,d]= sum_j p[i,j,b,n] v[j,b,n,d]

Because the 1e30*mask shift is constant along the softmax axis n, masked
(i,j,b) entries become exactly -1e30 for ALL n (score absorbed by fp32
rounding) -> softmax yields exactly uniform 1/16.  Unmasked entries get a
plain softmax-over-n of the raw scaled scores.  So exactly:

    p = (1-m) * softmax_n(s) + m * (1/16)

Kernel strategy (8 cores = 2 batches x 4 i-chunks of 512):
    per core, per i-block of 256 (x2), per j-group of 512 (4 j-tiles of 128):
      scores   S^T[j,i] per n = K_n^T.T @ Q_n^T      (PE, fp16, N=256)
      E        = exp(0.125 * S)                       (ACT, psum->sbuf, fp16)
      Z        = sum_n E   (pairwise tree)            (DVE, fp16 2x mode)
      W        = 1/Z                                  (DVE reciprocal approx)
      C        = W * (1-m)^T                          (DVE)
      P_n      = E_n * C       (in place)             (DVE)
      out[i,n,:] += P_n-tile-as-lhsT @ V_n  +  (m/16)-tile @ V_n  (PE, fp16)
    transposed AV: out tiles are [128 i, 64 d] = 256B/partition per head, so
    all 16 heads' accumulators fit in 4 PSUM banks and stay resident across
    every j-group (no per-group merges; one ACT evacuation per i-block;
    natural [i,n,d] output).  Interleaved per-head accumulation in shared
    banks is safe: start=True only on each bank's first matmul; every other
    head's first touch relies on has_written=0 -> overwrite semantics.

E is triple-buffered at j-group granularity, and scores/exp of block g+1
are issued jt-by-jt interleaved with the AV matmuls of block g so the
in-order PE stream never stalls behind the DVE softmax chain.

Engine balance (HW-measured, not CoreSim): Pool/GpSimd tensor ops are far
slower on silicon than the sim models (~15us/head lost for any normalize
head placed there, broadcast or dense) — the optimum is ALL elementwise
work on DVE (HSPL=16) with Pool doing only SWDGE DMA issue, and the
PSUM->SBUF output evac on ACT (EVAC_ACT).  Measured 110.6us/rep vs the
261.9us baseline on the same differential-KREPS harness; the kernel sits
at the ACT exp floor (16.8M exps/core at 1 elem/lane/cycle @1.2GHz).

All input layout transforms (transposes / (1-m) / m/16 / fp16 casts) are done
host-side in numpy so every device DMA is a linear copy.
"""

import sys

import numpy as np

if "/opt/trn_rl_repo" not in sys.path:
    sys.path.insert(0, "/opt/trn_rl_repo")

SEQ, B, N, D = 2048, 2, 16, 64
SCALE = 0.125
NCORES = 8
IC = 512          # i-chunk per core (SEQ / 4 i-chunks)
IB = 256          # i columns per inner block
NIB = IC // IB    # 2
JT = 128          # j per tile (partition dim)
JG = 4            # j-tiles per j-group
NJG = SEQ // (JT * JG)   # 4 groups
NG = 4            # heads per score-psum group
HSPL = 16         # heads normalized on DVE; rest on GpSimd/Pool (HW A/B:
                  # any tensor op on Pool/Q7 loses ~15us/head vs DVE — Pool
                  # does only DMA issue in the final config)
EVAC_ACT = True   # PSUM->SBUF output evacuation on ACT (scalar) vs DVE
ODMA_SP = False   # both output DMAs on the sync HWDGE queue (off Pool/SWDGE)
TREE_POOL = False # run tree level-1 (heads 0-7 pairs) on Pool instead of DVE
RECIP_ACT = False # 1/Z on the scalar engine instead of DVE custom op
MMFP8 = False     # mask (m/16)@V matmuls in fp8e4m3 DoubleRow (2x PE rate;
                  # m/16 exact in e4m3, fp8 V adds ~1e-2 rel err)
EXPDVE = ()       # ng score groups (0..3) whose exp runs on DVE via the
                  # Schraudolph int16 trick instead of ACT (offloads the
                  # scalar engine at ~2% rel err on those heads)
# fp16 Schraudolph constants: bits(exp(x)) ~= round(x*2^10*log2e + 15*2^10 - adj)
# with x = SCALE*s; folding SCALE: t = s*184.6650 + 15301.0
EXP_C1 = float(np.float32(SCALE * 1024.0 / np.log(2.0)))
EXP_C2 = 15301.0

_CACHE = {}

import os
KREPS = int(os.environ.get("KREPS", "1"))  # repeat pipeline for differential timing


def _build_nc():
    import concourse.bacc as bacc
    import concourse.mybir as mybir
    import concourse.tile as tile

    dt = mybir.dt
    f32, f16 = dt.float32, dt.float16
    Alu = mybir.AluOpType

    nc = bacc.Bacc("TRN2", target_bir_lowering=False, debug=False)

    # Per-core inputs (host pre-laid-out so every DMA is linear):
    #  qT : [64 d, 16 n, 512 i]             fp16  Q^T of this core's i-chunk
    #  kT : [16 jg, 64 d, 16 n, 128 j]      fp16  K^T tiles
    #  vp : [128 p, 16 n, 16 jg, 64 d]      fp16  V with j=jg*128+p
    #  at : [16 jt, 2 ib, 128 p, 256 i]    fp16  (1-m)^T tiles
    #  mt : same layout                     fp16  (m/16)^T tiles
    qT = nc.dram_tensor("qT", [D, N, IC], f16, kind="ExternalInput").ap()
    kT = nc.dram_tensor("kT", [SEQ // JT, D, N, JT], f16, kind="ExternalInput").ap()
    vp = nc.dram_tensor("vp", [JT, N, SEQ // JT, D], f16, kind="ExternalInput").ap()
    at = nc.dram_tensor("at", [SEQ // JT, NIB, JT, IB], f16, kind="ExternalInput").ap()
    if MMFP8:
        f8 = dt.float8e4
        mt8 = nc.dram_tensor(
            "mt8", [SEQ // JT, NIB, JT // 2, 2, IB], f8, kind="ExternalInput"
        ).ap()
        vp8 = nc.dram_tensor(
            "vp8", [JT // 2, 2, N, SEQ // JT, D], f8, kind="ExternalInput"
        ).ap()
    else:
        mt = nc.dram_tensor(
            "mt", [SEQ // JT, NIB, JT, IB], f16, kind="ExternalInput"
        ).ap()
    # natural [i, n, d] output layout (transposed-AV writes i on partitions)
    out = nc.dram_tensor("out", [IC, N, D], f32, kind="ExternalOutput").ap()

    with tile.TileContext(nc) as tc:
        with (
            tc.tile_pool(name="consts", bufs=1) as consts,
            tc.tile_pool(name="kstream", bufs=4) as kpool,
            tc.tile_pool(name="epool", bufs=(2 if MMFP8 else 3)) as epool,
            tc.tile_pool(name="zpool", bufs=1) as zpool,
            tc.tile_pool(name="maskp", bufs=2) as maskp,
            tc.tile_pool(name="opool", bufs=2) as opool,
            tc.tile_pool(name="spsum", bufs=2, space="PSUM") as spsum,
            tc.tile_pool(name="rpsum", bufs=1, space="PSUM") as rpsum,
        ):
            q_sb = consts.tile([D, N, IC], f16)
            # split by head-group so the first score matmuls (heads 0-3)
            # are unblocked after ~1/4 of the q transfer
            nc.sync.dma_start(q_sb[:, 0:4, 0:IB], qT[:, 0:4, 0:IB])
            for ng in range(1, 4):
                nc.gpsimd.dma_start(
                    q_sb[:, ng * 4:(ng + 1) * 4, 0:IB],
                    qT[:, ng * 4:(ng + 1) * 4, 0:IB],
                )
            v_sb = consts.tile([JT, N, SEQ // JT, D], f16)
            if MMFP8:
                v8_sb = consts.tile([JT // 2, 2, N, SEQ // JT, D], dt.float8e4)

            # PE warm-up: the HAM clock gate holds the PE at 1.2 GHz until
            # ~3.4us of sustained activity.  Burn dummy matmuls during the
            # initial DMA wait so the real scores start at full clock.
            wu = consts.tile([D, IB], f16, name="wu")
            nc.vector.memset(wu[:], 0.5)
            wups = spsum.tile([JT, NG, IB], f32, tag="s", name="wu_ps")
            for w in range(16):
                nc.tensor.matmul(
                    wups[:, 0, :], wu[:, 0:JT], wu[:],
                    start=True, stop=True, skip_group_check=True,
                )

            o_sbs = {}

            def scores_jt(ib, jg, jt, E, h, post_ng=None):
                """PE scores + ACT exp for one head-half of one j-tile.
                Called h-OUTER (all jts' heads 0-7 before any heads 8-15) so
                the softmax tree's first level can start after half the
                exps of the block instead of nearly all of them.  post_ng
                is invoked after each 4-head score group so AV work can be
                woven in at sub-exp granularity."""
                k_sb = kpool.tile([D, N // 2, JT], f16, tag="k",
                                  name=f"k_{jg}_{ib}_{h}")
                nc.sync.dma_start(
                    k_sb[:], kT[jg, :, h * (N // 2):(h + 1) * (N // 2), :]
                )
                for ng2 in range(N // NG // 2):
                    ng = h * 2 + ng2
                    ps = spsum.tile([JT, NG, IB], f32, tag="s",
                                    name=f"ps_{jg}_{ib}_{ng}")
                    for nn in range(NG):
                        n_ = ng * NG + nn
                        nc.tensor.matmul(
                            ps[:, nn, :],
                            k_sb[:, ng2 * NG + nn, :],
                            q_sb[:, n_, ib * IB:(ib + 1) * IB],
                            start=True,
                            stop=True,
                        )
                    if ng in EXPDVE:
                        with nc.allow_low_precision("schraudolph fast exp"):
                            nc.vector.tensor_scalar(
                                E[:, jt, ng * NG:(ng + 1) * NG, :].bitcast(
                                    dt.int16),
                                ps[:], EXP_C1, EXP_C2, Alu.mult, Alu.add,
                            )
                    else:
                        nc.scalar.activation(
                            E[:, jt, ng * NG:(ng + 1) * NG, :],
                            ps[:],
                            mybir.ActivationFunctionType.Exp,
                            scale=SCALE,
                        )
                    if post_ng is not None:
                        post_ng()

            def tree_alloc(blkid):
                return zpool.tile([JT, JG, 6, IB], f16, tag="zb", bufs=2,
                                  name=f"zb_{blkid}")

            def tree_a(E, zb_t, jlo, jhi):
                """First tree levels for heads 0-7 of j-tiles [jlo,jhi).
                Emittable as soon as those tiles' h=0 exps are done.  L1 can
                run on Pool (TREE_POOL): it is dense (no broadcast, Q7's
                cheaper mode) and has the most slack — produced during the
                h=0 score pass, consumed only by softmax_half."""
                zb = zb_t[:, jlo:jhi]
                Ev = E[:, jlo:jhi]
                with nc.allow_low_precision("softmax partial sums in fp16"):
                    ev = Ev.rearrange("p j (h two) i -> p j two h i", two=2)
                    eng1 = nc.gpsimd if TREE_POOL else nc.vector
                    eng1.tensor_tensor(
                        zb[:, :, 0:4], ev[:, :, 0, 0:4], ev[:, :, 1, 0:4], Alu.add
                    )
                    za = zb[:, :, 0:4].rearrange("p j (h two) i -> p j two h i", two=2)
                    nc.vector.tensor_tensor(
                        zb[:, :, 0:2], za[:, :, 0], za[:, :, 1], Alu.add
                    )

            def softmax_half(E, a_sb, zb_t, blkid, jlo, jhi,
                             use_pool=True):
                """DVE: finish the Z tree for j-tiles [jlo,jhi) (heads 8-15
                levels + combine), reciprocal, C; then normalize E in place —
                heads [0:HSPL) on DVE, heads [HSPL:16) on GpSimd/Pool in
                per-j-tile chunks (keeps the chain latency ahead of the AV
                matmuls that consume E)."""
                w = jhi - jlo
                zb = zb_t[:, jlo:jhi]
                zf_t = zpool.tile([JT, JG, IB], f32, tag="zf", bufs=2,
                                  name=f"zf_{blkid}_{jlo}")
                zf = zf_t[:, jlo:jhi]
                cc_t = zpool.tile([JT, JG, IB], f16, tag="cc", bufs=2,
                                  name=f"cc_{blkid}_{jlo}")
                cc = cc_t[:, jlo:jhi]
                Ev = E[:, jlo:jhi]
                with nc.allow_low_precision("softmax partial sums in fp16"):
                    ev = Ev.rearrange("p j (h two) i -> p j two h i", two=2)
                    nc.vector.tensor_tensor(
                        zb[:, :, 2:6], ev[:, :, 0, 4:8], ev[:, :, 1, 4:8], Alu.add
                    )
                    zc = zb[:, :, 2:6].rearrange("p j (h two) i -> p j two h i", two=2)
                    nc.vector.tensor_tensor(
                        zb[:, :, 2:4], zc[:, :, 0], zc[:, :, 1], Alu.add
                    )
                    nc.vector.tensor_tensor(
                        zb[:, :, 0:2], zb[:, :, 0:2], zb[:, :, 2:4], Alu.add
                    )
                    nc.vector.tensor_tensor(
                        zf[:], zb[:, :, 0], zb[:, :, 1], Alu.add
                    )
                if RECIP_ACT:
                    # emit ACT Reciprocal directly (bass guards it behind a
                    # ValueError for accuracy; our 2e-2 budget tolerates it,
                    # and it moves ~10us off the critical DVE chain)
                    eng = nc.scalar
                    ins_ = [eng.lower_ap(zf[:])]
                    for v in (0.0, 1.0, 0.0):  # bias, scale, alpha
                        ins_.append(mybir.ImmediateValue(
                            dtype=mybir.dt.float32, value=v))
                    eng.add_instruction(mybir.InstActivation(
                        name=eng.bass.get_next_instruction_name(),
                        func=mybir.ActivationFunctionType.Reciprocal,
                        ins=ins_,
                        outs=[eng.lower_ap(zf[:])],
                    ))
                else:
                    nc.vector.reciprocal_approx_fast(out=zf[:], in_=zf[:])
                nc.vector.tensor_tensor(cc[:], zf[:], a_sb[:, jlo:jhi], Alu.mult)
                hs = HSPL if use_pool else N
                nc.vector.tensor_tensor(
                    Ev[:, :, 0:hs, :],
                    Ev[:, :, 0:hs, :],
                    cc[:, :, None, :].to_broadcast((JT, w, hs, IB)),
                    Alu.mult,
                )
                if use_pool and HSPL < N:
                    for jj in range(jlo, jhi):
                        nc.gpsimd.tensor_tensor(
                            E[:, jj, HSPL:N, :],
                            E[:, jj, HSPL:N, :],
                            cc_t[:, jj, None, :].to_broadcast(
                                (JT, N - HSPL, IB)),
                            Alu.mult,
                        )

            av_started = {}

            def av_chunk(blk, pj, ih, stop_now):
                """Transposed AV for one (j-tile, i-half) of a previous
                block: ~860ns of PE work, woven between score groups so the
                in-order PE stream never delays the exp feed by more than
                ACT's slack."""
                rep, ib, jg0, sz, first, last, E, m_sb, accs = blk[:9]
                acc = accs[ih]
                key = (rep, ib, ih)
                started = av_started.get(key, False)
                for n_ in range(N):
                    nc.tensor.matmul(
                        acc[:, n_, :],
                        E[:, pj, n_, ih * 128:(ih + 1) * 128],
                        v_sb[:, n_, jg0 + pj, :],
                        start=(not started and n_ % 8 == 0),
                        stop=False,
                        skip_group_check=True,
                    )
                av_started[key] = True
                for nh in range(2):
                    # one 512-col matmul covers 8 heads' acc regions
                    # (exactly one PSUM bank); rhs is V for heads
                    # nh*8..nh*8+7 of this j-tile, 2D-strided AP.
                    if MMFP8:
                        nc.tensor.matmul(
                            acc[:, nh * 8:(nh + 1) * 8, :],
                            m_sb[:, :, pj, ih * 128:(ih + 1) * 128],
                            v8_sb[:, :, nh * 8:(nh + 1) * 8, jg0 + pj, :],
                            start=False,
                            stop=stop_now,
                            skip_group_check=True,
                            perf_mode=mybir.MatmulPerfMode.DoubleRow,
                        )
                    else:
                        nc.tensor.matmul(
                            acc[:, nh * 8:(nh + 1) * 8, :],
                            m_sb[:, pj, ih * 128:(ih + 1) * 128],
                            v_sb[:, nh * 8:(nh + 1) * 8, jg0 + pj, :],
                            start=False,
                            stop=stop_now,
                            skip_group_check=True,
                        )

            def evac_ih(blk, ih):
                """One PSUM accumulator -> SBUF (DVE) -> DRAM [i,n,d].
                Emitted right after that accumulator's final AV chunk; the
                copy runs on DVE to keep ACT exclusively on the exp stream."""
                rep, ib, accs = blk[0], blk[1], blk[8]
                o_sb = opool.tile([JT, N, D], f32, tag="o",
                                  name=f"o_{rep}_{ib}_{ih}")
                if EVAC_ACT:
                    nc.scalar.copy(o_sb[:], accs[ih][:])
                else:
                    nc.vector.tensor_scalar(
                        o_sb[:], accs[ih][:], 1.0, 0.0, Alu.mult, Alu.add
                    )
                i0 = ib * IB + ih * JT
                if ih == 0 or ODMA_SP:
                    nc.sync.dma_start(out[i0:i0 + JT], o_sb[:])
                else:
                    nc.gpsimd.dma_start(out[i0:i0 + JT], o_sb[:])

            # Software pipeline: the scores/exp of block g+1 are interleaved
            # jt-by-jt with the AV matmuls of block g, so the in-order PE
            # stream never stalls behind the DVE softmax chain.
            prev = None
            vload = 0
            for rep in range(KREPS):
              for ib in range(NIB):
                last_ib = rep == KREPS - 1 and ib == NIB - 1
                SIZES = [4, 4, 4, 4] if last_ib else [4, 4, 4, 4]
                acc0 = rpsum.tile([JT, N, D], f32, tag="acc0",
                                  name=f"acc_{rep}_{ib}_0")
                acc1 = rpsum.tile([JT, N, D], f32, tag="acc1",
                                  name=f"acc_{rep}_{ib}_1")
                accs = [acc0, acc1]
                jg0 = 0
                for bi, sz in enumerate(SIZES):
                    a_sb = maskp.tile([JT, JG, IB], f16, tag="a", bufs=2,
                                      name=f"a_{rep}_{ib}_{bi}")
                    if MMFP8:
                        m_sb = maskp.tile([JT // 2, 2, JG, IB], dt.float8e4,
                                          tag="m", name=f"m_{rep}_{ib}_{bi}")
                    else:
                        m_sb = maskp.tile([JT, JG, IB], f16, tag="m",
                                          name=f"m_{rep}_{ib}_{bi}")
                    E = epool.tile([JT, JG, N, IB], f16, tag="E",
                                   name=f"E_{rep}_{ib}_{bi}")
                    blkid = f"{rep}_{ib}_{bi}"
                    zb_t = None
                    # Previous block's AV work, as (j-tile, i-half) chunks in
                    # the order the Pool normalize completes them (prev's h=1
                    # processing order), scheduled one chunk per score group
                    # starting late enough that each chunk's softmax chain
                    # is finished when the in-order PE reaches it.
                    if prev is not None:
                        chunks = [(pj, ihh) for pj in prev[9]
                                  for ihh in (0, 1)]
                    else:
                        chunks = []
                    nslots = 4 * sz
                    slot_start = max(2, nslots - 3 - len(chunks))
                    st = {"slot": 0, "ci": 0}

                    def post_ng(prev=prev, chunks=chunks, st=st,
                                slot_start=slot_start):
                        st["slot"] += 1
                        if st["ci"] < len(chunks) and st["slot"] >= slot_start:
                            ci = st["ci"]
                            pj, ihh = chunks[ci]
                            av_chunk(prev, pj, ihh,
                                     prev[5] and ci >= len(chunks) - 2)
                            st["ci"] += 1
                            if prev[5] and ci >= len(chunks) - 2:
                                evac_ih(prev, ihh)

                    # h=0 pass: scores for heads 0-7 of every j-tile
                    # (ascending); the first tree level of each 2-tile half
                    # is emitted as soon as its exps are queued.
                    for jt in range(sz):
                        scores_jt(ib, jg0 + jt, jt, E, 0, post_ng)
                        if jt == 0:
                            # mask DMAs issued after the block's first k DMA
                            # so they don't head-of-line-block the k stream
                            # on the SP queue
                            nc.sync.dma_start(
                                a_sb[:, :sz],
                                at[jg0:jg0 + sz, ib].rearrange("j p i -> p j i"),
                            )
                            if MMFP8:
                                nc.sync.dma_start(
                                    m_sb[:, :, :sz],
                                    mt8[jg0:jg0 + sz, ib].rearrange(
                                        "j p t i -> p t j i"),
                                )
                            else:
                                nc.sync.dma_start(
                                    m_sb[:, :sz],
                                    mt[jg0:jg0 + sz, ib].rearrange("j p i -> p j i"),
                                )
                        if rep == 0 and vload < 4 and ib == 0:
                            nc.gpsimd.dma_start(
                                v_sb[:, :, vload * 4:(vload + 1) * 4, :],
                                vp[:, :, vload * 4:(vload + 1) * 4, :],
                            )
                            vload += 1
                            if vload == 4 and MMFP8:
                                nc.gpsimd.dma_start(v8_sb[:], vp8[:])
                            if vload == 4:
                                nc.gpsimd.dma_start(
                                    q_sb[:, :, IB:IC], qT[:, :, IB:IC]
                                )
                        if jt % 2 == 1 or jt == sz - 1:
                            if zb_t is None:
                                zb_t = tree_alloc(blkid)
                            tree_a(E, zb_t, (jt // 2) * 2, jt + 1)
                    # h=1 pass: scores for heads 8-15, j-tiles in order
                    # [2,3,0,1] so the second half's softmax chain (and its
                    # Pool normalize chunks) complete first, matching the AV
                    # chunk consumption order of the NEXT block.
                    h1ord = [2, 3, 0, 1] if sz == 4 else list(range(sz))
                    done = set()
                    for jt in h1ord:
                        scores_jt(ib, jg0 + jt, jt, E, 1, post_ng)
                        done.add(jt)
                        half = (jt // 2) * 2
                        hhi = min(half + 2, sz)
                        if all(j in done for j in range(half, hhi)):
                            softmax_half(E, a_sb, zb_t, blkid, half, hhi,
                                         use_pool=(sz == 4))
                    # flush any AV chunks that didn't fit in this block's
                    # score slots (small trailing blocks)
                    while st["ci"] < len(chunks):
                        ci = st["ci"]
                        pj, ihh = chunks[ci]
                        av_chunk(prev, pj, ihh,
                                 prev[5] and ci >= len(chunks) - 2)
                        st["ci"] += 1
                        if prev[5] and ci >= len(chunks) - 2:
                            evac_ih(prev, ihh)
                    prev = (rep, ib, jg0, sz, bi == 0, bi == len(SIZES) - 1,
                            E, m_sb, accs, h1ord)
                    jg0 += sz
            tail = [(pj, ihh) for pj in prev[9] for ihh in (0, 1)]
            for ci, (pj, ihh) in enumerate(tail):
                av_chunk(prev, pj, ihh, ci >= len(tail) - 2)
                if ci >= len(tail) - 2:
                    evac_ih(prev, ihh)

    nc.compile()
    return nc


def _prep_core_inputs(q_head, k_head, v_head, attn_mask):
    """Host-side shard + layout. Returns list of 8 per-core input dicts."""
    in_maps = []
    for c in range(NCORES):
        b = c // 4
        i0 = (c % 4) * IC
        q = q_head[i0:i0 + IC, b]                      # [512, 16, 64] fp32
        k = k_head[:, b]                               # [2048, 16, 64]
        v = v_head[:, b]                               # [2048, 16, 64]
        m = attn_mask[i0:i0 + IC, :, b, 0]             # [512 i, 2048 j]

        qTc = np.ascontiguousarray(q.transpose(2, 1, 0)).astype(np.float16)
        kTc = np.ascontiguousarray(
            k.reshape(SEQ // JT, JT, N, D).transpose(0, 3, 2, 1)
        ).astype(np.float16)                                    # [16, 64, 16, 128]
        vpc = np.ascontiguousarray(
            v.reshape(SEQ // JT, JT, N, D).transpose(1, 2, 0, 3)
        ).astype(np.float16)                                    # [128, 16, 16, 64]
        A = np.ascontiguousarray((1.0 - m).T)                   # [2048 j, 512 i]
        M = np.ascontiguousarray(m.T) * np.float32(1.0 / 16.0)
        atc = np.ascontiguousarray(
            A.reshape(SEQ // JT, JT, NIB, IB).transpose(0, 2, 1, 3)
        ).astype(np.float16)                                    # [16, 2, 128, 256]
        im = {"qT": qTc, "kT": kTc, "vp": vpc, "at": atc}
        if MMFP8:
            from concourse import mybir
            f8np = mybir.dt.np(mybir.dt.float8e4)
            # j_local = t*64 + p packing, shared by mask and V operands
            im["mt8"] = np.ascontiguousarray(
                M.reshape(SEQ // JT, 2, JT // 2, NIB, IB)
                .transpose(0, 3, 2, 1, 4)
            ).astype(f8np)                                      # [16,2,64,2,256]
            im["vp8"] = np.ascontiguousarray(
                v.reshape(SEQ // JT, 2, JT // 2, N, D)
                .transpose(2, 1, 3, 0, 4)
            ).astype(f8np)                                      # [64,2,16,16,64]
        else:
            im["mt"] = np.ascontiguousarray(
                M.reshape(SEQ // JT, JT, NIB, IB).transpose(0, 2, 1, 3)
            ).astype(np.float16)
        in_maps.append(im)
    return in_maps


def run_on_cores(q_head, k_head, v_head, attn_mask, trace=False, **kw):
    from concourse.bass_utils import run_bass_kernel_spmd

    if "nc" not in _CACHE:
        _CACHE["nc"] = _build_nc()
    nc = _CACHE["nc"]
    in_maps = _prep_core_inputs(q_head, k_head, v_head, attn_mask)
    res = run_bass_kernel_spmd(
        nc, in_maps, core_ids=list(range(NCORES)), trace=trace, **kw
    )
    outs = np.empty((SEQ, B, N, D), dtype=np.float32)
    for c in range(NCORES):
        b = c // 4
        i0 = (c % 4) * IC
        outs[i0:i0 + IC, b] = res.results[c]["out"]
    return outs, res


def kernel(q_head, k_head, v_head, attn_mask):
    out, _ = run_on_cores(
        np.asarray(q_head, dtype=np.float32),
        np.asarray(k_head, dtype=np.float32),
        np.asarray(v_head, dtype=np.float32),
        np.asarray(attn_mask, dtype=np.float32),
    )
    return out

